# revision 29
# baseline (speedup 1.0000x reference)
"""BitConv2d (ternary-quantized 3x3 conv) on 8 Trainium2 NeuronCores.

Contract: kernel(**inputs) takes FULL unsharded inputs
  x [32, 256, 56, 56] f32, weight [256, 256, 3, 3] f32, bias [256] f32,
  scale_ema scalar f32
and returns the FULL output y [32, 256, 56, 56] f32.

Strategy: data-parallel over batch (4 images / core), weights replicated.
  Pass 1 (device): per-core max(|x_shard|) -> host combine -> beta.
  Host: quantize weights (tiny), cast to fp8 e4m3, fold scalars.
  Pass 2 (device): quantize x to an EXACT pair of fp8 planes
        (a = 8*rne(q/8) in multiples of 8 <= 128, b = q - a in [-4,4];
        both exactly representable in e4m3), then 3x3 conv via fp8
        DoubleRow matmuls: each pair (w8, w8) x (a, b) contributes
        w8 * (a + b) = w8 * q exactly, so the only quantization error
        vs the reference is the fp8 rounding of the weights (~1e-2 rel).

  DoubleRow halves the PE time per K-block vs fp16.  The padded images
  are stored as flat 58-wide rows so that every 3x3 tap window of an
  8-row output strip is a single contiguous 464-element slice (the
  horizontal pad columns absorb the row-wrap); output columns 0 and 57
  are junk and are simply not copied out.
"""

import numpy as np

import concourse.bass as bass
import concourse.tile as tile
from concourse import bacc, mybir
from concourse.bass_interp import get_hw_module
from concourse.bass_utils import run_bass_kernel_spmd

_NCORES = 8
_M1 = 12582912.0  # 1.5 * 2**23: adding+subtracting forces round-to-nearest-even
_F32 = mybir.dt.float32
_F16 = mybir.dt.float16
_F8 = mybir.dt.float8e4

# padded-image layout: 58 rows x 58 cols flat + slack, 16B aligned
_WP = 58
_IMG = 3376  # 58*58 = 3364 data + 12 slack
_BASE = 1    # image data starts at offset 1 (guard elem at 0 for tap offset -1)
# image 0 is split into 4 padded-row sub-blocks (2 strips each) so the
# first matmuls only wait for a quarter of its quantize
_B0 = [(0, 17), (16, 33), (32, 49), (48, 57)]   # padrow ranges per block
_BIMG = 1056  # 18*58 = 1044 data + 12 slack, 16-aligned

# results of the last kernel() call, for test.py introspection
last_results = {}

# debug: ablate parts of the conv kernel for timeline-sim analysis
# (must stay empty in production; cache key includes it)
_ABLATE = frozenset()
_WARMUP_MMS = 90
_IMG0_SPREAD = False
_C0_FIRST = True


def _build_max_kernel(nsh, cin, h, w):
    """Per-core abs-max over the x shard -> mx [128,1] (partition partials)."""
    nc = bacc.Bacc("TRN2", target_bir_lowering=False, debug=False,
                   num_devices=_NCORES)
    x = nc.dram_tensor("x", [nsh, cin, h, w], _F32, kind="ExternalInput")
    mx = nc.dram_tensor("mx", [128, 1], _F32, kind="ExternalOutput")
    cinc = cin // 128
    # quarter-chunk granularity so the final reduce tail is short
    # quarter chunks; the final image-chunk tapers into 196-element pieces
    # so the post-DMA reduce chain is short
    hwq = (h * w) // 4
    plan = []
    for n in range(nsh):
        for c in range(cinc):
            if n == nsh - 1 and c == cinc - 1:
                plan.append((n, c, [hwq, hwq, hwq] + [hwq // 4] * 4))
            else:
                plan.append((n, c, [hwq] * 4))
    ntiles = sum(len(p[2]) for p in plan)
    with tile.TileContext(nc, trace_sim=False) as tc:
        with tc.tile_pool(name="xs", bufs=4) as xs, \
             tc.tile_pool(name="acc", bufs=1) as accp:
            pm = accp.tile([128, ntiles], _F32)
            k = 0
            for n, c, sizes in plan:
                xt = xs.tile([128, h * w], _F32, name="xt", tag="xt")
                off = 0
                for sz in sizes:
                    sl = xt[:, off:off + sz]
                    nc.sync.dma_start(
                        sl, x.ap()[n, c * 128:(c + 1) * 128]
                        .rearrange("p a b -> p (a b)")[:, off:off + sz])
                    nc.vector.reduce_max(pm[:, k:k + 1], sl,
                                         axis=mybir.AxisListType.X,
                                         apply_absolute_value=True)
                    off += sz
                    k += 1
            mxt = accp.tile([128, 1], _F32)
            nc.vector.reduce_max(mxt[:], pm[:], axis=mybir.AxisListType.X)
            nc.sync.dma_start(mx.ap(), mxt[:])
    nc.compile()
    nc.m = get_hw_module(nc.m)
    return nc


def _build_conv_kernel(nsh, cin, cout, h, w):
    """Quantize x (exact fp8 pair) + 3x3 same-pad conv, fp8 DoubleRow.

    Inputs per core:
      x  [nsh, cin, h, w] f32
      wq [coc, 128, 4608] f8   (per-co-half lhsT pair blocks, partition-major)
      b  [cout//128, 128, 1] f32
      sc [128, 2] f32          (inv_beta, beta*gamma)
    Output: y [nsh, cout, h, w] f32
    """
    assert h == 56 and w == 56
    cinc, coc = cin // 128, cout // 128
    rowg = h // 8                      # 8-row output strips per image

    nc = bacc.Bacc("TRN2", target_bir_lowering=False, debug=False,
                   num_devices=_NCORES)
    x = nc.dram_tensor("x", [nsh, cin, h, w], _F32, kind="ExternalInput")
    wq = nc.dram_tensor("wq", [coc, 128, 9 * cinc * 2 * 128], _F8,
                        kind="ExternalInput")
    b = nc.dram_tensor("b", [coc, 128, 1], _F32, kind="ExternalInput")
    sc = nc.dram_tensor("sc", [128, 2], _F32, kind="ExternalInput")
    y = nc.dram_tensor("y", [nsh, cout, h, w], _F32, kind="ExternalOutput")

    Ident = mybir.ActivationFunctionType.Identity
    DR = mybir.MatmulPerfMode.DoubleRow

    with tile.TileContext(nc, trace_sim=False) as tc:
        with tc.tile_pool(name="const", bufs=1) as const, \
             tc.tile_pool(name="xstage", bufs=3) as xstage, \
             tc.tile_pool(name="outs", bufs=3) as outs, \
             tc.tile_pool(name="psum", bufs=8, space="PSUM") as psum:

            # ---- constants -------------------------------------------------
            # preload the ACT function table (lazy-load costs 1.3us on the
            # first activation otherwise)
            scratch = const.tile([128, 1], _F32)
            nc.scalar.activation(scratch[:],
                                 nc.const_aps.tensor(0.0, (128, 1)), Ident)
            # warm the PE while the head DMAs run (cost model p-state ramp)
            zw = const.tile([128, 128], _F16)
            nc.vector.memset(zw[:], 0.0)
            psw = psum.tile([128, 128], _F32, name="psw", tag="ps")
            for _ in range(_WARMUP_MMS):
                nc.tensor.matmul(psw[:], zw[:], zw[:], start=True, stop=True)

            w_sbs = [const.tile([128, 9, cinc, 2, 128], _F8,
                                name=f"w_sb{i}") for i in range(coc)]
            sc_sb = const.tile([128, 2], _F32)
            b_sb = const.tile([128, coc], _F32)
            mg_p = const.tile([128, 1], _F32)
            nc.vector.memset(mg_p[:], _M1)
            # tiny scalars lead the SWDGE queue (ACT needs sc first)
            nc.gpsimd.dma_start(sc_sb[:], sc.ap())
            nc.gpsimd.dma_start(b_sb[:], b.ap().rearrange("c p o -> p (c o)"))
            # first co-half of weights leads the DMA device (gates strip 0)
            nc.gpsimd.dma_start(w_sbs[0][:], wq.ap()[0])

            # ---- padded quantized pair planes (fp8, zero borders) ----------
            # image 0 lives in hl0 as 4 row sub-blocks (2 strips each) so the
            # first matmuls only wait for a quarter of its quantize; images
            # 1..nsh-1 are whole padded images in hl.  pair planes adjacent
            # so an MM's rhs byte-interval only spans its own image block.
            hl = const.tile([128, cinc, nsh - 1, 2, _IMG], _F8)
            hl0 = const.tile([128, cinc, len(_B0), 2, _BIMG], _F8)

            def _borders(img2, rows, top_pad, bot_pad, eng):
                """img2: [128, 2, block] pair-consolidated view."""
                if top_pad:
                    eng.memset(img2[:, :, 0:_BASE + _WP], 0.0)
                else:
                    eng.memset(img2[:, :, 0:_BASE + 1], 0.0)
                if bot_pad:
                    eng.memset(img2[:, :, _BASE + (rows - 1) * _WP:], 0.0)
                else:
                    eng.memset(img2[:, :, _BASE + rows * _WP - 1:], 0.0)
                eng.memset(
                    img2[:, :, _BASE + 57:_BASE + 57 + (rows - 1) * _WP]
                    .rearrange("p two (r t) -> p two r t", t=_WP)[:, :, :, 0:2],
                    0.0)

            # image-0 block borders up front (cheap, gate the first strips)
            mi = 0
            for c in range(cinc):
                for bi, (plo, phi) in enumerate(_B0):
                    eng = nc.vector if mi % 2 == 0 else nc.gpsimd
                    mi += 1
                    _borders(hl0[:, c, bi, :, :], phi - plo + 1,
                             plo == 0, phi == 57, eng)

            def _img2d(pair, c, n):
                return hl[:, c, n - 1, pair, _BASE:_BASE + 3364] \
                    .rearrange("p (r t) -> p r t", t=_WP)

            def _blk2d(pair, c, bi):
                plo, phi = _B0[bi]
                rows = phi - plo + 1
                return hl0[:, c, bi, pair, _BASE:_BASE + rows * _WP] \
                    .rearrange("p (r t) -> p r t", t=_WP)

            # x_q = round_half_even(x * inv_beta); |x*inv_beta| < 127 so no
            # clip needed.  Exact fp8 split: a8 = fp8(q) (RNE to <=4-bit
            # significand), b8 = q - a8 (integer in [-4,4], exact in fp8).
            qi = 0
            for n in range(nsh):
                if n >= 1:
                    # borders for this image (needed before its a/b writes)
                    for c in range(cinc):
                        eng = nc.vector if c == 0 else nc.gpsimd
                        _borders(hl[:, c, n - 1, :, :], _WP, True, True, eng)
                xts = [xstage.tile([128, h, w], _F32, name="xt", tag="xt")
                       for _ in range(cinc)]
                nch = {0: 4, 1: 2}.get(n, 1)
                rch = h // nch
                iters = []
                for r in range(0, h, rch):
                    for c in range(cinc):
                        nc.sync.dma_start(
                            xts[c][:, r:r + rch, :],
                            x.ap()[n, c * 128:(c + 1) * 128, r:r + rch, :])
                        iters.append((r, rch, c))
                    if n == 1 and r == 0:
                        nc.gpsimd.dma_start(w_sbs[1][:], wq.ap()[1])
                if True:
                    for r, rch, c in iters:
                        xsl = xts[c][:, r:r + rch, :]
                        # t = x*invb + M1 (ACT), q = t - M1 (DVE)
                        nc.scalar.activation(xsl, xsl, Ident,
                                             bias=mg_p[:],
                                             scale=sc_sb[:, 0:1])
                        nc.vector.tensor_scalar(xsl, xsl, -_M1, None,
                                                op0=mybir.AluOpType.add)
                        # destination slices: whole image (n>=1) or the
                        # sub-blocks intersecting this chunk's padrows (n=0)
                        if n == 0:
                            dests = []
                            for bi, (plo, phi) in enumerate(_B0):
                                lo = max(r + 1, plo)
                                hi = min(r + rch, phi)
                                if lo > hi:
                                    continue
                                ll, cl = lo - plo, lo - (r + 1)
                                nr = hi - lo + 1
                                dests.append((
                                    _blk2d(0, c, bi)[:, ll:ll + nr, 1:57],
                                    _blk2d(1, c, bi)[:, ll:ll + nr, 1:57],
                                    cl, nr))
                        else:
                            dests = [(
                                _img2d(0, c, n)[:, 1 + r:1 + r + rch, 1:57],
                                _img2d(1, c, n)[:, 1 + r:1 + r + rch, 1:57],
                                0, rch)]
                        for a_sl, b_sl, cl, nr in dests:
                            # a8 = fp8-RNE(q) (DVE)
                            nc.vector.tensor_scalar(
                                a_sl, xsl[:, cl:cl + nr, :], 0.0, None,
                                op0=mybir.AluOpType.add)
                            # b8 = q - a (mixed f32/fp8 in, fp8 out).
                            # Pool's t_t is ~3x slower than DVE's, so put
                            # the gate-critical image-0 c0 parts on DVE.
                            if n == 0:
                                b_eng = nc.vector if c == 0 else nc.gpsimd
                            else:
                                b_eng = nc.vector if qi % 4 == 3 \
                                    else nc.gpsimd
                            b_eng.tensor_tensor(
                                b_sl, xsl[:, cl:cl + nr, :], a_sl,
                                op=mybir.AluOpType.subtract)
                        qi += 1

            # ---- conv: 18 DoubleRow matmuls per [128co x nr x 58w] strip --
            def _mm_strip_c(ps, c, r0, co, start, stop, n=0, nr=8):
                if n == 0:
                    bi = min(r0 // 16, len(_B0) - 1)
                    lr0 = r0 - _B0[bi][0]
                for tap in range(9):
                    dh, dw = tap // 3, tap % 3
                    w_ap = w_sbs[co][:, tap, c, :, :]
                    if n == 0:
                        o = (lr0 + dh) * _WP + dw
                        rhs = hl0[:, c, bi, :, o:o + nr * _WP]
                    else:
                        o = (r0 + dh) * _WP + dw
                        rhs = hl[:, c, n - 1, :, o:o + nr * _WP]
                    nc.tensor.matmul(ps[:], w_ap, rhs,
                                     start=(start and tap == 0),
                                     stop=(stop and tap == 8),
                                     perf_mode=DR)

            def _mm_strip(ps, n, r0, co, nr=8):
                for c in range(cinc):
                    _mm_strip_c(ps, c, r0, co, start=(c == 0),
                                stop=(c == cinc - 1), n=n, nr=nr)

            for n in range(nsh):
                for co in range(coc):
                    ost = outs.tile([128, h, w], _F32, name="ost", tag="ost")
                    pss = []
                    if n == 0 and co == 0:
                        # first tile: issue all strips' c0 groups first so
                        # the PE has work while c1's quantize finishes
                        for s in range(rowg):
                            ps = psum.tile([128, 8, _WP], _F32, name="ps",
                                           tag="ps")
                            _mm_strip_c(ps, 0, 8 * s, co, start=True,
                                        stop=False)
                            pss.append(ps)
                    # the very last strip is split into 4-row halves so
                    # the post-last-matmul epilogue+DMA tail is short
                    strips = [(8 * s, 8) for s in range(rowg)]
                    if n == nsh - 1 and co == coc - 1:
                        strips = strips[:-1] + [(48, 4), (52, 4)]
                    for s, (r0, snr) in enumerate(strips):
                        if pss:
                            ps = pss[s]
                            _mm_strip_c(ps, 1, r0, co, start=False,
                                        stop=True)
                        else:
                            ps = psum.tile([128, snr, _WP], _F32, name="ps",
                                           tag="ps")
                            _mm_strip(ps, n, r0, co, nr=snr)
                        # epilogue: beta*gamma*acc + bias, drop junk
                        # cols 0 and 57; alternate ACT/DVE per strip
                        if s % 2 == 0:
                            nc.scalar.activation(ost[:, r0:r0 + snr, :],
                                                 ps[:, :, 1:57], Ident,
                                                 bias=b_sb[:, co:co + 1],
                                                 scale=sc_sb[:, 1:2])
                        else:
                            nc.vector.tensor_scalar(
                                ost[:, r0:r0 + snr, :], ps[:, :, 1:57],
                                sc_sb[:, 1:2], b_sb[:, co:co + 1],
                                op0=mybir.AluOpType.mult,
                                op1=mybir.AluOpType.add)
                    # whole-image DMAs keep HBM descriptors large (>=512B)
                    nhalf = 8 if (n == nsh - 1 and co == coc - 1) else 2
                    hh = h // nhalf
                    for k in range(nhalf):
                        nc.sync.dma_start(
                            y.ap()[n, co * 128:(co + 1) * 128,
                                   k * hh:(k + 1) * hh, :],
                            ost[:, k * hh:(k + 1) * hh, :])
    nc.compile()
    nc.m = get_hw_module(nc.m)
    return nc


_cache = {}


def _get(builder, *args):
    key = (builder.__name__,) + args + (tuple(sorted(_ABLATE)),)
    if key not in _cache:
        _cache[key] = builder(*args)
    return _cache[key]


def _run(nc, in_maps, cores):
    """run_bass_kernel_spmd with retries for transient device errors."""
    import time
    last = None
    for attempt in range(3):
        try:
            return run_bass_kernel_spmd(nc, in_maps, cores)
        except Exception as e:
            last = e
            time.sleep(2.0 * (attempt + 1))
    raise last


def _quantize_weights(weight, gamma):
    """Bit-exact f32 replication of the reference chimera-ternary transform."""
    f32 = np.float32
    ws = (weight / gamma).astype(f32)
    tern = np.clip(np.round(ws), f32(-1.0), f32(1.0)).astype(f32)
    raw = (f32(1.0 - 0.7) * ws + f32(0.7) * tern).astype(f32)
    # straight-through estimator is an fp identity only up to rounding:
    # replicate w + (raw - w) op-for-op, then clamp
    ste = (weight + (raw - weight)).astype(f32)
    return np.clip(ste, f32(-1.0), f32(1.0)).astype(f32)


def kernel(x, weight, bias, scale_ema):
    import ml_dtypes
    x = np.ascontiguousarray(x, dtype=np.float32)
    weight = np.ascontiguousarray(weight, dtype=np.float32)
    bias = np.ascontiguousarray(bias, dtype=np.float32)
    f32 = np.float32
    N, cin, h, w = x.shape
    cout = weight.shape[0]
    nsh = N // _NCORES
    cores = list(range(_NCORES))

    # ---- host-side tiny prep (beta-independent, done before launch 1 so
    # the gap between the two device launches is only scalar math) ---------
    gamma = np.maximum(f32(scale_ema), f32(1e-6))
    wqf = _quantize_weights(weight, gamma)
    # fp8 weights: [cout, cin, 3, 3] -> per-co-half partition-major lhsT
    # blocks [coc, ci(128), (tap, ci_chunk, pair, co_l)], pairs (w8, w8)
    w8 = wqf.astype(ml_dtypes.float8_e4m3)
    wl = np.ascontiguousarray(
        w8.transpose(2, 3, 1, 0).reshape(9, cin // 128, 128, cout))
    wp = np.stack([wl, wl], axis=3)          # [9, cinc, 128, 2, cout]
    wql = np.ascontiguousarray(
        np.stack([wp[:, :, :, :, co * 128:(co + 1) * 128]
                  .transpose(2, 0, 1, 3, 4)     # [128, 9, cinc, 2, 128]
                  .reshape(128, -1)
                  for co in range(cout // 128)], axis=0))
    b_l = np.ascontiguousarray(bias.reshape(cout // 128, 128, 1))
    ncA = _get(_build_max_kernel, nsh, cin, h, w)
    ncB = _get(_build_conv_kernel, nsh, cin, cout, h, w)

    # ---- pass 1: global abs-max -> beta ---------------------------------
    resA = _run(ncA, [{"x": x[i * nsh:(i + 1) * nsh]} for i in cores], cores)
    last_results["max"] = resA
    gmax = f32(max(f32(r["mx"].max()) for r in resA.results))
    beta = gmax / f32(127.0) + f32(1e-6)
    sc = np.tile(np.array([f32(1.0) / beta, beta * gamma], f32), (128, 1))
    sc = np.ascontiguousarray(sc)

    # ---- pass 2: quantize x + conv --------------------------------------
    in_maps = [{"x": x[i * nsh:(i + 1) * nsh], "wq": wql, "b": b_l, "sc": sc}
               for i in cores]
    resB = _run(ncB, in_maps, cores)
    last_results["conv"] = resB
    return np.concatenate([resB.results[i]["y"] for i in cores], axis=0)


# revision 30
# speedup vs baseline: 1.0057x; 1.0057x over previous
"""BitConv2d (ternary-quantized 3x3 conv) on 8 Trainium2 NeuronCores.

Contract: kernel(**inputs) takes FULL unsharded inputs
  x [32, 256, 56, 56] f32, weight [256, 256, 3, 3] f32, bias [256] f32,
  scale_ema scalar f32
and returns the FULL output y [32, 256, 56, 56] f32.

Strategy: data-parallel over batch (4 images / core), weights replicated.
  Pass 1 (device): per-core max(|x_shard|) -> host combine -> beta.
  Host: quantize weights (tiny), cast to fp8 e4m3, fold scalars.
  Pass 2 (device): quantize x to an EXACT pair of fp8 planes
        (a = 8*rne(q/8) in multiples of 8 <= 128, b = q - a in [-4,4];
        both exactly representable in e4m3), then 3x3 conv via fp8
        DoubleRow matmuls: each pair (w8, w8) x (a, b) contributes
        w8 * (a + b) = w8 * q exactly, so the only quantization error
        vs the reference is the fp8 rounding of the weights (~1e-2 rel).

  DoubleRow halves the PE time per K-block vs fp16.  The padded images
  are stored as flat 58-wide rows so that every 3x3 tap window of an
  8-row output strip is a single contiguous 464-element slice (the
  horizontal pad columns absorb the row-wrap); output columns 0 and 57
  are junk and are simply not copied out.
"""

import numpy as np

import concourse.bass as bass
import concourse.tile as tile
from concourse import bacc, mybir
from concourse.bass_interp import get_hw_module
from concourse.bass_utils import run_bass_kernel_spmd

_NCORES = 8
_M1 = 12582912.0  # 1.5 * 2**23: adding+subtracting forces round-to-nearest-even
_F32 = mybir.dt.float32
_F16 = mybir.dt.float16
_F8 = mybir.dt.float8e4

# padded-image layout: 58 rows x 58 cols flat + slack, 16B aligned
_WP = 58
_IMG = 3376  # 58*58 = 3364 data + 12 slack
_BASE = 1    # image data starts at offset 1 (guard elem at 0 for tap offset -1)
# image 0 is split into 4 padded-row sub-blocks (2 strips each) so the
# first matmuls only wait for a quarter of its quantize
_B0 = [(0, 17), (16, 33), (32, 49), (48, 57)]   # padrow ranges per block
_BIMG = 1056  # 18*58 = 1044 data + 12 slack, 16-aligned

# results of the last kernel() call, for test.py introspection
last_results = {}

# debug: ablate parts of the conv kernel for timeline-sim analysis
# (must stay empty in production; cache key includes it)
_ABLATE = frozenset()
_WARMUP_MMS = 105
_IMG0_SPREAD = False
_C0_FIRST = True


def _build_max_kernel(nsh, cin, h, w):
    """Per-core abs-max over the x shard -> mx [128,1] (partition partials)."""
    nc = bacc.Bacc("TRN2", target_bir_lowering=False, debug=False,
                   num_devices=_NCORES)
    x = nc.dram_tensor("x", [nsh, cin, h, w], _F32, kind="ExternalInput")
    mx = nc.dram_tensor("mx", [128, 1], _F32, kind="ExternalOutput")
    cinc = cin // 128
    # quarter-chunk granularity so the final reduce tail is short
    # quarter-chunk granularity so the final reduce tail is short
    nq = 4
    hwq = (h * w) // nq
    ntiles = nsh * cinc * nq
    with tile.TileContext(nc, trace_sim=False) as tc:
        with tc.tile_pool(name="xs", bufs=4) as xs, \
             tc.tile_pool(name="acc", bufs=1) as accp:
            pm = accp.tile([128, ntiles], _F32)
            k = 0
            for n in range(nsh):
                for c in range(cinc):
                    xt = xs.tile([128, h * w], _F32, name="xt", tag="xt")
                    for q in range(nq):
                        sl = xt[:, q * hwq:(q + 1) * hwq]
                        nc.sync.dma_start(
                            sl, x.ap()[n, c * 128:(c + 1) * 128]
                            .rearrange("p a b -> p (a b)")
                            [:, q * hwq:(q + 1) * hwq])
                        nc.vector.reduce_max(pm[:, k:k + 1], sl,
                                             axis=mybir.AxisListType.X,
                                             apply_absolute_value=True)
                        k += 1
            mxt = accp.tile([128, 1], _F32)
            nc.vector.reduce_max(mxt[:], pm[:], axis=mybir.AxisListType.X)
            nc.sync.dma_start(mx.ap(), mxt[:])
    nc.compile()
    nc.m = get_hw_module(nc.m)
    return nc


def _build_conv_kernel(nsh, cin, cout, h, w):
    """Quantize x (exact fp8 pair) + 3x3 same-pad conv, fp8 DoubleRow.

    Inputs per core:
      x  [nsh, cin, h, w] f32
      wq [coc, 128, 4608] f8   (per-co-half lhsT pair blocks, partition-major)
      b  [cout//128, 128, 1] f32
      sc [128, 2] f32          (inv_beta, beta*gamma)
    Output: y [nsh, cout, h, w] f32
    """
    assert h == 56 and w == 56
    cinc, coc = cin // 128, cout // 128
    rowg = h // 8                      # 8-row output strips per image

    nc = bacc.Bacc("TRN2", target_bir_lowering=False, debug=False,
                   num_devices=_NCORES)
    x = nc.dram_tensor("x", [nsh, cin, h, w], _F32, kind="ExternalInput")
    wq = nc.dram_tensor("wq", [coc, 128, 9 * cinc * 2 * 128], _F8,
                        kind="ExternalInput")
    b = nc.dram_tensor("b", [coc, 128, 1], _F32, kind="ExternalInput")
    sc = nc.dram_tensor("sc", [128, 2], _F32, kind="ExternalInput")
    y = nc.dram_tensor("y", [nsh, cout, h, w], _F32, kind="ExternalOutput")

    Ident = mybir.ActivationFunctionType.Identity
    DR = mybir.MatmulPerfMode.DoubleRow

    with tile.TileContext(nc, trace_sim=False) as tc:
        with tc.tile_pool(name="const", bufs=1) as const, \
             tc.tile_pool(name="xstage", bufs=3) as xstage, \
             tc.tile_pool(name="outs", bufs=3) as outs, \
             tc.tile_pool(name="psum", bufs=8, space="PSUM") as psum:

            # ---- constants -------------------------------------------------
            # preload the ACT function table (lazy-load costs 1.3us on the
            # first activation otherwise)
            scratch = const.tile([128, 1], _F32)
            nc.scalar.activation(scratch[:],
                                 nc.const_aps.tensor(0.0, (128, 1)), Ident)
            # warm the PE while the head DMAs run (cost model p-state ramp)
            zw = const.tile([128, 128], _F16)
            nc.vector.memset(zw[:], 0.0)
            psw = psum.tile([128, 128], _F32, name="psw", tag="ps")
            for _ in range(_WARMUP_MMS):
                nc.tensor.matmul(psw[:], zw[:], zw[:], start=True, stop=True)

            w_sbs = [const.tile([128, 9, cinc, 2, 128], _F8,
                                name=f"w_sb{i}") for i in range(coc)]
            sc_sb = const.tile([128, 2], _F32)
            b_sb = const.tile([128, coc], _F32)
            mg_p = const.tile([128, 1], _F32)
            nc.vector.memset(mg_p[:], _M1)
            # tiny scalars lead the SWDGE queue (ACT needs sc first)
            nc.gpsimd.dma_start(sc_sb[:], sc.ap())
            nc.gpsimd.dma_start(b_sb[:], b.ap().rearrange("c p o -> p (c o)"))
            # first co-half of weights leads the DMA device (gates strip 0)
            nc.gpsimd.dma_start(w_sbs[0][:], wq.ap()[0])

            # ---- padded quantized pair planes (fp8, zero borders) ----------
            # image 0 lives in hl0 as 4 row sub-blocks (2 strips each) so the
            # first matmuls only wait for a quarter of its quantize; images
            # 1..nsh-1 are whole padded images in hl.  pair planes adjacent
            # so an MM's rhs byte-interval only spans its own image block.
            hl = const.tile([128, cinc, nsh - 1, 2, _IMG], _F8)
            hl0 = const.tile([128, cinc, len(_B0), 2, _BIMG], _F8)

            def _borders(img2, rows, top_pad, bot_pad, eng):
                """img2: [128, 2, block] pair-consolidated view."""
                if top_pad:
                    eng.memset(img2[:, :, 0:_BASE + _WP], 0.0)
                else:
                    eng.memset(img2[:, :, 0:_BASE + 1], 0.0)
                if bot_pad:
                    eng.memset(img2[:, :, _BASE + (rows - 1) * _WP:], 0.0)
                else:
                    eng.memset(img2[:, :, _BASE + rows * _WP - 1:], 0.0)
                eng.memset(
                    img2[:, :, _BASE + 57:_BASE + 57 + (rows - 1) * _WP]
                    .rearrange("p two (r t) -> p two r t", t=_WP)[:, :, :, 0:2],
                    0.0)

            # image-0 block borders up front (cheap, gate the first strips)
            mi = 0
            for c in range(cinc):
                for bi, (plo, phi) in enumerate(_B0):
                    eng = nc.vector if mi % 2 == 0 else nc.gpsimd
                    mi += 1
                    _borders(hl0[:, c, bi, :, :], phi - plo + 1,
                             plo == 0, phi == 57, eng)

            def _img2d(pair, c, n):
                return hl[:, c, n - 1, pair, _BASE:_BASE + 3364] \
                    .rearrange("p (r t) -> p r t", t=_WP)

            def _blk2d(pair, c, bi):
                plo, phi = _B0[bi]
                rows = phi - plo + 1
                return hl0[:, c, bi, pair, _BASE:_BASE + rows * _WP] \
                    .rearrange("p (r t) -> p r t", t=_WP)

            # x_q = round_half_even(x * inv_beta); |x*inv_beta| < 127 so no
            # clip needed.  Exact fp8 split: a8 = fp8(q) (RNE to <=4-bit
            # significand), b8 = q - a8 (integer in [-4,4], exact in fp8).
            qi = 0
            for n in range(nsh):
                if n >= 1:
                    # borders for this image (needed before its a/b writes)
                    for c in range(cinc):
                        eng = nc.vector if c == 0 else nc.gpsimd
                        _borders(hl[:, c, n - 1, :, :], _WP, True, True, eng)
                xts = [xstage.tile([128, h, w], _F32, name="xt", tag="xt")
                       for _ in range(cinc)]
                nch = {0: 4, 1: 2}.get(n, 1)
                rch = h // nch
                iters = []
                for r in range(0, h, rch):
                    for c in range(cinc):
                        nc.sync.dma_start(
                            xts[c][:, r:r + rch, :],
                            x.ap()[n, c * 128:(c + 1) * 128, r:r + rch, :])
                        iters.append((r, rch, c))
                    if n == 1 and r == 0:
                        nc.gpsimd.dma_start(w_sbs[1][:], wq.ap()[1])
                if True:
                    for r, rch, c in iters:
                        xsl = xts[c][:, r:r + rch, :]
                        # t = x*invb + M1 (ACT), q = t - M1 (DVE)
                        nc.scalar.activation(xsl, xsl, Ident,
                                             bias=mg_p[:],
                                             scale=sc_sb[:, 0:1])
                        nc.vector.tensor_scalar(xsl, xsl, -_M1, None,
                                                op0=mybir.AluOpType.add)
                        # destination slices: whole image (n>=1) or the
                        # sub-blocks intersecting this chunk's padrows (n=0)
                        if n == 0:
                            dests = []
                            for bi, (plo, phi) in enumerate(_B0):
                                lo = max(r + 1, plo)
                                hi = min(r + rch, phi)
                                if lo > hi:
                                    continue
                                ll, cl = lo - plo, lo - (r + 1)
                                nr = hi - lo + 1
                                dests.append((
                                    _blk2d(0, c, bi)[:, ll:ll + nr, 1:57],
                                    _blk2d(1, c, bi)[:, ll:ll + nr, 1:57],
                                    cl, nr))
                        else:
                            dests = [(
                                _img2d(0, c, n)[:, 1 + r:1 + r + rch, 1:57],
                                _img2d(1, c, n)[:, 1 + r:1 + r + rch, 1:57],
                                0, rch)]
                        for a_sl, b_sl, cl, nr in dests:
                            # a8 = fp8-RNE(q) (DVE)
                            nc.vector.tensor_scalar(
                                a_sl, xsl[:, cl:cl + nr, :], 0.0, None,
                                op0=mybir.AluOpType.add)
                            # b8 = q - a (mixed f32/fp8 in, fp8 out).
                            # Pool's t_t is ~3x slower than DVE's, so put
                            # the gate-critical image-0 c0 parts on DVE.
                            if n == 0:
                                b_eng = nc.vector if c == 0 else nc.gpsimd
                            else:
                                b_eng = nc.vector if qi % 4 == 3 \
                                    else nc.gpsimd
                            b_eng.tensor_tensor(
                                b_sl, xsl[:, cl:cl + nr, :], a_sl,
                                op=mybir.AluOpType.subtract)
                        qi += 1

            # ---- conv: 18 DoubleRow matmuls per [128co x nr x 58w] strip --
            def _mm_strip_c(ps, c, r0, co, start, stop, n=0, nr=8):
                if n == 0:
                    bi = min(r0 // 16, len(_B0) - 1)
                    lr0 = r0 - _B0[bi][0]
                for tap in range(9):
                    dh, dw = tap // 3, tap % 3
                    w_ap = w_sbs[co][:, tap, c, :, :]
                    if n == 0:
                        o = (lr0 + dh) * _WP + dw
                        rhs = hl0[:, c, bi, :, o:o + nr * _WP]
                    else:
                        o = (r0 + dh) * _WP + dw
                        rhs = hl[:, c, n - 1, :, o:o + nr * _WP]
                    nc.tensor.matmul(ps[:], w_ap, rhs,
                                     start=(start and tap == 0),
                                     stop=(stop and tap == 8),
                                     perf_mode=DR)

            def _mm_strip(ps, n, r0, co, nr=8):
                for c in range(cinc):
                    _mm_strip_c(ps, c, r0, co, start=(c == 0),
                                stop=(c == cinc - 1), n=n, nr=nr)

            for n in range(nsh):
                for co in range(coc):
                    ost = outs.tile([128, h, w], _F32, name="ost", tag="ost")
                    pss = []
                    if n == 0 and co == 0:
                        # first tile: issue all strips' c0 groups first so
                        # the PE has work while c1's quantize finishes
                        for s in range(rowg):
                            ps = psum.tile([128, 8, _WP], _F32, name="ps",
                                           tag="ps")
                            _mm_strip_c(ps, 0, 8 * s, co, start=True,
                                        stop=False)
                            pss.append(ps)
                    # the very last strip is split into 4-row halves so
                    # the post-last-matmul epilogue+DMA tail is short
                    strips = [(8 * s, 8) for s in range(rowg)]
                    if n == nsh - 1 and co == coc - 1:
                        strips = strips[:-1] + [(48, 4), (52, 4)]
                    for s, (r0, snr) in enumerate(strips):
                        if pss:
                            ps = pss[s]
                            _mm_strip_c(ps, 1, r0, co, start=False,
                                        stop=True)
                        else:
                            ps = psum.tile([128, snr, _WP], _F32, name="ps",
                                           tag="ps")
                            _mm_strip(ps, n, r0, co, nr=snr)
                        # epilogue: beta*gamma*acc + bias, drop junk
                        # cols 0 and 57; alternate ACT/DVE per strip
                        if s % 2 == 0:
                            nc.scalar.activation(ost[:, r0:r0 + snr, :],
                                                 ps[:, :, 1:57], Ident,
                                                 bias=b_sb[:, co:co + 1],
                                                 scale=sc_sb[:, 1:2])
                        else:
                            nc.vector.tensor_scalar(
                                ost[:, r0:r0 + snr, :], ps[:, :, 1:57],
                                sc_sb[:, 1:2], b_sb[:, co:co + 1],
                                op0=mybir.AluOpType.mult,
                                op1=mybir.AluOpType.add)
                    # whole-image DMAs keep HBM descriptors large (>=512B)
                    nhalf = 8 if (n == nsh - 1 and co == coc - 1) else 2
                    hh = h // nhalf
                    for k in range(nhalf):
                        nc.sync.dma_start(
                            y.ap()[n, co * 128:(co + 1) * 128,
                                   k * hh:(k + 1) * hh, :],
                            ost[:, k * hh:(k + 1) * hh, :])
    nc.compile()
    nc.m = get_hw_module(nc.m)
    return nc


_cache = {}


def _get(builder, *args):
    key = (builder.__name__,) + args + (tuple(sorted(_ABLATE)),)
    if key not in _cache:
        _cache[key] = builder(*args)
    return _cache[key]


def _run(nc, in_maps, cores):
    """run_bass_kernel_spmd with retries for transient device errors."""
    import time
    last = None
    for attempt in range(3):
        try:
            return run_bass_kernel_spmd(nc, in_maps, cores)
        except Exception as e:
            last = e
            time.sleep(2.0 * (attempt + 1))
    raise last


def _quantize_weights(weight, gamma):
    """Bit-exact f32 replication of the reference chimera-ternary transform."""
    f32 = np.float32
    ws = (weight / gamma).astype(f32)
    tern = np.clip(np.round(ws), f32(-1.0), f32(1.0)).astype(f32)
    raw = (f32(1.0 - 0.7) * ws + f32(0.7) * tern).astype(f32)
    # straight-through estimator is an fp identity only up to rounding:
    # replicate w + (raw - w) op-for-op, then clamp
    ste = (weight + (raw - weight)).astype(f32)
    return np.clip(ste, f32(-1.0), f32(1.0)).astype(f32)


def kernel(x, weight, bias, scale_ema):
    import ml_dtypes
    x = np.ascontiguousarray(x, dtype=np.float32)
    weight = np.ascontiguousarray(weight, dtype=np.float32)
    bias = np.ascontiguousarray(bias, dtype=np.float32)
    f32 = np.float32
    N, cin, h, w = x.shape
    cout = weight.shape[0]
    nsh = N // _NCORES
    cores = list(range(_NCORES))

    # ---- host-side tiny prep (beta-independent, done before launch 1 so
    # the gap between the two device launches is only scalar math) ---------
    gamma = np.maximum(f32(scale_ema), f32(1e-6))
    wqf = _quantize_weights(weight, gamma)
    # fp8 weights: [cout, cin, 3, 3] -> per-co-half partition-major lhsT
    # blocks [coc, ci(128), (tap, ci_chunk, pair, co_l)], pairs (w8, w8)
    w8 = wqf.astype(ml_dtypes.float8_e4m3)
    wl = np.ascontiguousarray(
        w8.transpose(2, 3, 1, 0).reshape(9, cin // 128, 128, cout))
    wp = np.stack([wl, wl], axis=3)          # [9, cinc, 128, 2, cout]
    wql = np.ascontiguousarray(
        np.stack([wp[:, :, :, :, co * 128:(co + 1) * 128]
                  .transpose(2, 0, 1, 3, 4)     # [128, 9, cinc, 2, 128]
                  .reshape(128, -1)
                  for co in range(cout // 128)], axis=0))
    b_l = np.ascontiguousarray(bias.reshape(cout // 128, 128, 1))
    ncA = _get(_build_max_kernel, nsh, cin, h, w)
    ncB = _get(_build_conv_kernel, nsh, cin, cout, h, w)

    # ---- pass 1: global abs-max -> beta ---------------------------------
    resA = _run(ncA, [{"x": x[i * nsh:(i + 1) * nsh]} for i in cores], cores)
    last_results["max"] = resA
    gmax = f32(max(f32(r["mx"].max()) for r in resA.results))
    beta = gmax / f32(127.0) + f32(1e-6)
    sc = np.tile(np.array([f32(1.0) / beta, beta * gamma], f32), (128, 1))
    sc = np.ascontiguousarray(sc)

    # ---- pass 2: quantize x + conv --------------------------------------
    in_maps = [{"x": x[i * nsh:(i + 1) * nsh], "wq": wql, "b": b_l, "sc": sc}
               for i in cores]
    resB = _run(ncB, in_maps, cores)
    last_results["conv"] = resB
    return np.concatenate([resB.results[i]["y"] for i in cores], axis=0)


# revision 44
# speedup vs baseline: 1.0081x; 1.0024x over previous
"""BitConv2d (ternary-quantized 3x3 conv) on 8 Trainium2 NeuronCores.

Contract: kernel(**inputs) takes FULL unsharded inputs
  x [32, 256, 56, 56] f32, weight [256, 256, 3, 3] f32, bias [256] f32,
  scale_ema scalar f32
and returns the FULL output y [32, 256, 56, 56] f32.

Strategy: data-parallel over batch (4 images / core), weights replicated.
  Pass 1 (device): per-core max(|x_shard|) -> host combine -> beta.
  Host: quantize weights (tiny), cast to fp8 e4m3, fold scalars.
  Pass 2 (device): quantize x to an EXACT pair of fp8 planes
        (a = 8*rne(q/8) in multiples of 8 <= 128, b = q - a in [-4,4];
        both exactly representable in e4m3), then 3x3 conv via fp8
        DoubleRow matmuls: each pair (w8, w8) x (a, b) contributes
        w8 * (a + b) = w8 * q exactly, so the only quantization error
        vs the reference is the fp8 rounding of the weights (~1e-2 rel).

  DoubleRow halves the PE time per K-block vs fp16.  The padded images
  are stored as flat 58-wide rows so that every 3x3 tap window of an
  8-row output strip is a single contiguous 464-element slice (the
  horizontal pad columns absorb the row-wrap); output columns 0 and 57
  are junk and are simply not copied out.
"""

import numpy as np

import concourse.bass as bass
import concourse.tile as tile
from concourse import bacc, mybir
from concourse.bass_interp import get_hw_module
from concourse.bass_utils import run_bass_kernel_spmd

_NCORES = 8
_M1 = 12582912.0  # 1.5 * 2**23: adding+subtracting forces round-to-nearest-even
_F32 = mybir.dt.float32
_F16 = mybir.dt.float16
_F8 = mybir.dt.float8e4

# padded-image layout: 58 rows x 58 cols flat + slack, 16B aligned
_WP = 58
_IMG = 3376  # 58*58 = 3364 data + 12 slack
_BASE = 1    # image data starts at offset 1 (guard elem at 0 for tap offset -1)
# image 0 is split into 4 padded-row sub-blocks (2 strips each) so the
# first matmuls only wait for a quarter of its quantize
_B0 = [(0, 9), (8, 25), (24, 41), (40, 57)]   # padrow ranges per block
_BIMG = 1056  # 18*58 = 1044 data + 12 slack, 16-aligned

# results of the last kernel() call, for test.py introspection
last_results = {}

# debug: ablate parts of the conv kernel for timeline-sim analysis
# (must stay empty in production; cache key includes it)
_ABLATE = frozenset()
_WARMUP_MMS = 105
_IMG0_SPREAD = False
_C0_FIRST = True


def _build_max_kernel(nsh, cin, h, w):
    """Per-core abs-max over the x shard -> mx [128,1] (partition partials)."""
    nc = bacc.Bacc("TRN2", target_bir_lowering=False, debug=False,
                   num_devices=_NCORES)
    x = nc.dram_tensor("x", [nsh, cin, h, w], _F32, kind="ExternalInput")
    mx = nc.dram_tensor("mx", [128, 1], _F32, kind="ExternalOutput")
    cinc = cin // 128
    # quarter-chunk granularity so the final reduce tail is short
    # quarter-chunk granularity so the final reduce tail is short
    nq = 4
    hwq = (h * w) // nq
    ntiles = nsh * cinc * nq
    with tile.TileContext(nc, trace_sim=False) as tc:
        with tc.tile_pool(name="xs", bufs=4) as xs, \
             tc.tile_pool(name="acc", bufs=1) as accp:
            pm = accp.tile([128, ntiles], _F32)
            k = 0
            for n in range(nsh):
                for c in range(cinc):
                    xt = xs.tile([128, h * w], _F32, name="xt", tag="xt")
                    for q in range(nq):
                        sl = xt[:, q * hwq:(q + 1) * hwq]
                        nc.sync.dma_start(
                            sl, x.ap()[n, c * 128:(c + 1) * 128]
                            .rearrange("p a b -> p (a b)")
                            [:, q * hwq:(q + 1) * hwq])
                        nc.vector.reduce_max(pm[:, k:k + 1], sl,
                                             axis=mybir.AxisListType.X,
                                             apply_absolute_value=True)
                        k += 1
            mxt = accp.tile([128, 1], _F32)
            nc.vector.reduce_max(mxt[:], pm[:], axis=mybir.AxisListType.X)
            nc.sync.dma_start(mx.ap(), mxt[:])
    nc.compile()
    nc.m = get_hw_module(nc.m)
    return nc


def _build_conv_kernel(nsh, cin, cout, h, w):
    """Quantize x (exact fp8 pair) + 3x3 same-pad conv, fp8 DoubleRow.

    Inputs per core:
      x  [nsh, cin, h, w] f32
      wq [coc, 128, 4608] f8   (per-co-half lhsT pair blocks, partition-major)
      b  [cout//128, 128, 1] f32
      sc [128, 2] f32          (inv_beta, beta*gamma)
    Output: y [nsh, cout, h, w] f32
    """
    assert h == 56 and w == 56
    cinc, coc = cin // 128, cout // 128
    rowg = h // 8                      # 8-row output strips per image

    nc = bacc.Bacc("TRN2", target_bir_lowering=False, debug=False,
                   num_devices=_NCORES)
    x = nc.dram_tensor("x", [nsh, cin, h, w], _F32, kind="ExternalInput")
    wq = nc.dram_tensor("wq", [coc, cinc, 128, 9 * 2 * 128], _F8,
                        kind="ExternalInput")
    b = nc.dram_tensor("b", [coc, 128, 1], _F32, kind="ExternalInput")
    sc = nc.dram_tensor("sc", [128, 2], _F32, kind="ExternalInput")
    y = nc.dram_tensor("y", [nsh, cout, h, w], _F32, kind="ExternalOutput")

    Ident = mybir.ActivationFunctionType.Identity
    DR = mybir.MatmulPerfMode.DoubleRow

    with tile.TileContext(nc, trace_sim=False) as tc:
        with tc.tile_pool(name="const", bufs=1) as const, \
             tc.tile_pool(name="xstage", bufs=3) as xstage, \
             tc.tile_pool(name="outs", bufs=3) as outs, \
             tc.tile_pool(name="psum", bufs=8, space="PSUM") as psum:

            # ---- constants -------------------------------------------------
            # preload the ACT function table (lazy-load costs 1.3us on the
            # first activation otherwise)
            scratch = const.tile([128, 1], _F32)
            nc.scalar.activation(scratch[:],
                                 nc.const_aps.tensor(0.0, (128, 1)), Ident)
            # warm the PE while the head DMAs run (cost model p-state ramp)
            zw = const.tile([128, 128], _F16)
            nc.vector.memset(zw[:], 0.0)
            psw = psum.tile([128, 128], _F32, name="psw", tag="ps")
            for _ in range(_WARMUP_MMS):
                nc.tensor.matmul(psw[:], zw[:], zw[:], start=True, stop=True)

            w_sbs = [[const.tile([128, 9, 2, 128], _F8,
                                 name=f"w_sb{i}_{c}") for c in range(cinc)]
                     for i in range(coc)]
            sc_sb = const.tile([128, 2], _F32)
            b_sb = const.tile([128, coc], _F32)
            mg_p = const.tile([128, 1], _F32)
            nc.vector.memset(mg_p[:], _M1)
            # tiny scalars lead the SWDGE queue (ACT needs sc first)
            nc.gpsimd.dma_start(sc_sb[:], sc.ap())
            nc.gpsimd.dma_start(b_sb[:], b.ap().rearrange("c p o -> p (c o)"))


            # ---- padded quantized pair planes (fp8, zero borders) ----------
            # image 0 lives in hl0 as 4 row sub-blocks (2 strips each) so the
            # first matmuls only wait for a quarter of its quantize; images
            # 1..nsh-1 are whole padded images in hl.  pair planes adjacent
            # so an MM's rhs byte-interval only spans its own image block.
            hl = const.tile([128, cinc, nsh - 1, 2, _IMG], _F8)
            hl0 = const.tile([128, cinc, len(_B0), 2, _BIMG], _F8)

            def _borders(img2, rows, top_pad, bot_pad, eng):
                """img2: [128, 2, block] pair-consolidated view."""
                if top_pad:
                    eng.memset(img2[:, :, 0:_BASE + _WP], 0.0)
                else:
                    eng.memset(img2[:, :, 0:_BASE + 1], 0.0)
                if bot_pad:
                    eng.memset(img2[:, :, _BASE + (rows - 1) * _WP:], 0.0)
                else:
                    eng.memset(img2[:, :, _BASE + rows * _WP - 1:], 0.0)
                eng.memset(
                    img2[:, :, _BASE + 57:_BASE + 57 + (rows - 1) * _WP]
                    .rearrange("p two (r t) -> p two r t", t=_WP)[:, :, :, 0:2],
                    0.0)

            # image-0 block borders up front (cheap, gate the first strips)
            mi = 0
            for c in range(cinc):
                for bi, (plo, phi) in enumerate(_B0):
                    eng = nc.vector if mi % 2 == 0 else nc.gpsimd
                    mi += 1
                    _borders(hl0[:, c, bi, :, :], phi - plo + 1,
                             plo == 0, phi == 57, eng)

            def _img2d(pair, c, n):
                return hl[:, c, n - 1, pair, _BASE:_BASE + 3364] \
                    .rearrange("p (r t) -> p r t", t=_WP)

            def _blk2d(pair, c, bi):
                plo, phi = _B0[bi]
                rows = phi - plo + 1
                return hl0[:, c, bi, pair, _BASE:_BASE + rows * _WP] \
                    .rearrange("p (r t) -> p r t", t=_WP)

            # x_q = round_half_even(x * inv_beta); |x*inv_beta| < 127 so no
            # clip needed.  Exact fp8 split: a8 = fp8(q) (RNE to <=4-bit
            # significand), b8 = q - a8 (integer in [-4,4], exact in fp8).
            qi = 0
            for n in range(nsh):
                if n >= 1:
                    # borders for this image (needed before its a/b writes)
                    for c in range(cinc):
                        eng = nc.vector if c == 0 else nc.gpsimd
                        _borders(hl[:, c, n - 1, :, :], _WP, True, True, eng)
                xts = [xstage.tile([128, h, w], _F32, name="xt", tag="xt")
                       for _ in range(cinc)]
                nch = {0: 4, 1: 2}.get(n, 1)
                rch = h // nch
                iters = []
                for r in range(0, h, rch):
                    for c in range(cinc):
                        nc.sync.dma_start(
                            xts[c][:, r:r + rch, :],
                            x.ap()[n, c * 128:(c + 1) * 128, r:r + rch, :])
                        iters.append((r, rch, c))
                        # co0 weights ride the sync queue: the c0-tap half
                        # right behind the first chunk (gates strip 0), the
                        # c1 half later (phase B starts much later)
                        if n == 0 and r == 0 and c == 0:
                            nc.sync.dma_start(w_sbs[0][0][:], wq.ap()[0, 0])
                        if n == 0 and r == rch and c == 0:
                            nc.sync.dma_start(w_sbs[0][1][:], wq.ap()[0, 1])
                    if n == 1 and r == 0:
                        for c2 in range(cinc):
                            nc.gpsimd.dma_start(w_sbs[1][c2][:],
                                                wq.ap()[1, c2])
                if True:
                    for r, rch, c in iters:
                        xsl = xts[c][:, r:r + rch, :]
                        # t = x*invb + M1 (ACT), q = t - M1 (DVE)
                        nc.scalar.activation(xsl, xsl, Ident,
                                             bias=mg_p[:],
                                             scale=sc_sb[:, 0:1])
                        nc.vector.tensor_scalar(xsl, xsl, -_M1, None,
                                                op0=mybir.AluOpType.add)
                        # destination slices: whole image (n>=1) or the
                        # sub-blocks intersecting this chunk's padrows (n=0)
                        if n == 0:
                            dests = []
                            for bi, (plo, phi) in enumerate(_B0):
                                lo = max(r + 1, plo)
                                hi = min(r + rch, phi)
                                if lo > hi:
                                    continue
                                ll, cl = lo - plo, lo - (r + 1)
                                nr = hi - lo + 1
                                dests.append((
                                    _blk2d(0, c, bi)[:, ll:ll + nr, 1:57],
                                    _blk2d(1, c, bi)[:, ll:ll + nr, 1:57],
                                    cl, nr))
                        else:
                            dests = [(
                                _img2d(0, c, n)[:, 1 + r:1 + r + rch, 1:57],
                                _img2d(1, c, n)[:, 1 + r:1 + r + rch, 1:57],
                                0, rch)]
                        for a_sl, b_sl, cl, nr in dests:
                            # a8 = fp8-RNE(q) (DVE)
                            nc.vector.tensor_scalar(
                                a_sl, xsl[:, cl:cl + nr, :], 0.0, None,
                                op0=mybir.AluOpType.add)
                            # b8 = q - a (mixed f32/fp8 in, fp8 out).
                            # Pool's t_t is ~3x slower than DVE's, so put
                            # the gate-critical image-0 c0 parts on DVE.
                            if n == 0:
                                b_eng = nc.vector if c == 0 else nc.gpsimd
                            else:
                                b_eng = nc.vector if qi % 4 == 3 \
                                    else nc.gpsimd
                            b_eng.tensor_tensor(
                                b_sl, xsl[:, cl:cl + nr, :], a_sl,
                                op=mybir.AluOpType.subtract)
                        qi += 1

            # ---- conv: 18 DoubleRow matmuls per [128co x nr x 58w] strip --
            def _mm_strip_c(ps, c, r0, co, start, stop, n=0, nr=8):
                if n == 0:
                    bi = min((r0 + 8) // 16, len(_B0) - 1)
                    lr0 = r0 - _B0[bi][0]
                for tap in range(9):
                    dh, dw = tap // 3, tap % 3
                    w_ap = w_sbs[co][c][:, tap, :, :]
                    if n == 0:
                        o = (lr0 + dh) * _WP + dw
                        rhs = hl0[:, c, bi, :, o:o + nr * _WP]
                    else:
                        o = (r0 + dh) * _WP + dw
                        rhs = hl[:, c, n - 1, :, o:o + nr * _WP]
                    nc.tensor.matmul(ps[:], w_ap, rhs,
                                     start=(start and tap == 0),
                                     stop=(stop and tap == 8),
                                     perf_mode=DR)

            def _mm_strip(ps, n, r0, co, nr=8):
                for c in range(cinc):
                    _mm_strip_c(ps, c, r0, co, start=(c == 0),
                                stop=(c == cinc - 1), n=n, nr=nr)

            for n in range(nsh):
                for co in range(coc):
                    ost = outs.tile([128, h, w], _F32, name="ost", tag="ost")
                    pss = []
                    if n == 0 and co == 0:
                        # first tile: issue all strips' c0 groups first so
                        # the PE has work while c1's quantize finishes
                        for s in range(rowg):
                            ps = psum.tile([128, 8, _WP], _F32, name="ps",
                                           tag="ps")
                            _mm_strip_c(ps, 0, 8 * s, co, start=True,
                                        stop=False)
                            pss.append(ps)
                    # the very last strip is split into 4-row halves so
                    # the post-last-matmul epilogue+DMA tail is short
                    strips = [(8 * s, 8) for s in range(rowg)]
                    if n == nsh - 1 and co == coc - 1:
                        strips = strips[:-1] + [(48, 4), (52, 4)]
                    for s, (r0, snr) in enumerate(strips):
                        if pss:
                            ps = pss[s]
                            _mm_strip_c(ps, 1, r0, co, start=False,
                                        stop=True)
                        else:
                            ps = psum.tile([128, snr, _WP], _F32, name="ps",
                                           tag="ps")
                            _mm_strip(ps, n, r0, co, nr=snr)
                        # epilogue: beta*gamma*acc + bias, drop junk
                        # cols 0 and 57; alternate ACT/DVE per strip
                        if s % 2 == 0:
                            nc.scalar.activation(ost[:, r0:r0 + snr, :],
                                                 ps[:, :, 1:57], Ident,
                                                 bias=b_sb[:, co:co + 1],
                                                 scale=sc_sb[:, 1:2])
                        else:
                            nc.vector.tensor_scalar(
                                ost[:, r0:r0 + snr, :], ps[:, :, 1:57],
                                sc_sb[:, 1:2], b_sb[:, co:co + 1],
                                op0=mybir.AluOpType.mult,
                                op1=mybir.AluOpType.add)
                    # whole-image DMAs keep HBM descriptors large (>=512B);
                    # the last image's DMAs align to strips so the final DMA
                    # covers only the last 4-row half-strip
                    if n == nsh - 1 and co == coc - 1:
                        cuts = [(8 * k, 8) for k in range(6)] + \
                               [(48, 4), (52, 4)]
                    else:
                        cuts = [(0, 28), (28, 28)]
                    for r0d, nrd in cuts:
                        nc.sync.dma_start(
                            y.ap()[n, co * 128:(co + 1) * 128,
                                   r0d:r0d + nrd, :],
                            ost[:, r0d:r0d + nrd, :])
    nc.compile()
    nc.m = get_hw_module(nc.m)
    return nc


_cache = {}


def _get(builder, *args):
    key = (builder.__name__,) + args + (tuple(sorted(_ABLATE)),)
    if key not in _cache:
        _cache[key] = builder(*args)
    return _cache[key]


def _run(nc, in_maps, cores):
    """run_bass_kernel_spmd with retries for transient device errors."""
    import time
    last = None
    for attempt in range(3):
        try:
            return run_bass_kernel_spmd(nc, in_maps, cores)
        except Exception as e:
            last = e
            time.sleep(2.0 * (attempt + 1))
    raise last


def _quantize_weights(weight, gamma):
    """Bit-exact f32 replication of the reference chimera-ternary transform."""
    f32 = np.float32
    ws = (weight / gamma).astype(f32)
    tern = np.clip(np.round(ws), f32(-1.0), f32(1.0)).astype(f32)
    raw = (f32(1.0 - 0.7) * ws + f32(0.7) * tern).astype(f32)
    # straight-through estimator is an fp identity only up to rounding:
    # replicate w + (raw - w) op-for-op, then clamp
    ste = (weight + (raw - weight)).astype(f32)
    return np.clip(ste, f32(-1.0), f32(1.0)).astype(f32)


def kernel(x, weight, bias, scale_ema):
    import ml_dtypes
    x = np.ascontiguousarray(x, dtype=np.float32)
    weight = np.ascontiguousarray(weight, dtype=np.float32)
    bias = np.ascontiguousarray(bias, dtype=np.float32)
    f32 = np.float32
    N, cin, h, w = x.shape
    cout = weight.shape[0]
    nsh = N // _NCORES
    cores = list(range(_NCORES))

    # ---- host-side tiny prep (beta-independent, done before launch 1 so
    # the gap between the two device launches is only scalar math) ---------
    gamma = np.maximum(f32(scale_ema), f32(1e-6))
    wqf = _quantize_weights(weight, gamma)
    # fp8 weights: [cout, cin, 3, 3] -> per-co-half partition-major lhsT
    # blocks [coc, ci(128), (tap, ci_chunk, pair, co_l)], pairs (w8, w8)
    w8 = wqf.astype(ml_dtypes.float8_e4m3)
    wl = np.ascontiguousarray(
        w8.transpose(2, 3, 1, 0).reshape(9, cin // 128, 128, cout))
    wp = np.stack([wl, wl], axis=3)          # [9, cinc, 128, 2, cout]
    wql = np.ascontiguousarray(
        np.stack([np.stack([wp[:, c, :, :, co * 128:(co + 1) * 128]
                            .transpose(1, 0, 2, 3)  # [128, 9, 2, 128]
                            .reshape(128, -1)
                            for c in range(cin // 128)], axis=0)
                  for co in range(cout // 128)], axis=0))
    b_l = np.ascontiguousarray(bias.reshape(cout // 128, 128, 1))
    ncA = _get(_build_max_kernel, nsh, cin, h, w)
    ncB = _get(_build_conv_kernel, nsh, cin, cout, h, w)

    # ---- pass 1: global abs-max -> beta ---------------------------------
    resA = _run(ncA, [{"x": x[i * nsh:(i + 1) * nsh]} for i in cores], cores)
    last_results["max"] = resA
    gmax = f32(max(f32(r["mx"].max()) for r in resA.results))
    beta = gmax / f32(127.0) + f32(1e-6)
    sc = np.tile(np.array([f32(1.0) / beta, beta * gamma], f32), (128, 1))
    sc = np.ascontiguousarray(sc)

    # ---- pass 2: quantize x + conv --------------------------------------
    in_maps = [{"x": x[i * nsh:(i + 1) * nsh], "wq": wql, "b": b_l, "sc": sc}
               for i in cores]
    resB = _run(ncB, in_maps, cores)
    last_results["conv"] = resB
    return np.concatenate([resB.results[i]["y"] for i in cores], axis=0)


# revision 54
# speedup vs baseline: 1.0104x; 1.0023x over previous
"""BitConv2d (ternary-quantized 3x3 conv) on 8 Trainium2 NeuronCores.

Contract: kernel(**inputs) takes FULL unsharded inputs
  x [32, 256, 56, 56] f32, weight [256, 256, 3, 3] f32, bias [256] f32,
  scale_ema scalar f32
and returns the FULL output y [32, 256, 56, 56] f32.

Strategy: data-parallel over batch (4 images / core), weights replicated.
  Pass 1 (device): per-core max(|x_shard|) -> host combine -> beta.
  Host: quantize weights (tiny), cast to fp8 e4m3, fold scalars.
  Pass 2 (device): quantize x to an EXACT pair of fp8 planes
        (a = 8*rne(q/8) in multiples of 8 <= 128, b = q - a in [-4,4];
        both exactly representable in e4m3), then 3x3 conv via fp8
        DoubleRow matmuls: each pair (w8, w8) x (a, b) contributes
        w8 * (a + b) = w8 * q exactly, so the only quantization error
        vs the reference is the fp8 rounding of the weights (~1e-2 rel).

  DoubleRow halves the PE time per K-block vs fp16.  The padded images
  are stored as flat 58-wide rows so that every 3x3 tap window of an
  8-row output strip is a single contiguous 464-element slice (the
  horizontal pad columns absorb the row-wrap); output columns 0 and 57
  are junk and are simply not copied out.
"""

import numpy as np

import concourse.bass as bass
import concourse.tile as tile
from concourse import bacc, mybir
from concourse.bass_interp import get_hw_module
from concourse.bass_utils import run_bass_kernel_spmd

_NCORES = 8
_M1 = 12582912.0  # 1.5 * 2**23: adding+subtracting forces round-to-nearest-even
_F32 = mybir.dt.float32
_F16 = mybir.dt.float16
_F8 = mybir.dt.float8e4

# padded-image layout: 58 rows x 58 cols flat + slack, 16B aligned
_WP = 58
_IMG = 3376  # 58*58 = 3364 data + 12 slack
_BASE = 1    # image data starts at offset 1 (guard elem at 0 for tap offset -1)
# image 0 is split into 4 padded-row sub-blocks (2 strips each) so the
# first matmuls only wait for a quarter of its quantize
_B0 = [(0, 9), (8, 25), (24, 41), (40, 57)]   # padrow ranges per block
_BIMG = 1056  # 18*58 = 1044 data + 12 slack, 16-aligned

# results of the last kernel() call, for test.py introspection
last_results = {}

# debug: ablate parts of the conv kernel for timeline-sim analysis
# (must stay empty in production; cache key includes it)
_ABLATE = frozenset()
_WARMUP_MMS = 105
_IMG0_SPREAD = False
_C0_FIRST = True


def _build_max_kernel(nsh, cin, h, w):
    """Per-core abs-max over the x shard -> mx [128,1] (partition partials)."""
    nc = bacc.Bacc("TRN2", target_bir_lowering=False, debug=False,
                   num_devices=_NCORES)
    x = nc.dram_tensor("x", [nsh, cin, h, w], _F32, kind="ExternalInput")
    mx = nc.dram_tensor("mx", [128, 1], _F32, kind="ExternalOutput")
    cinc = cin // 128
    # quarter-chunk granularity so the final reduce tail is short
    # quarter-chunk granularity so the final reduce tail is short
    nq = 4
    hwq = (h * w) // nq
    ntiles = nsh * cinc * nq
    with tile.TileContext(nc, trace_sim=False) as tc:
        with tc.tile_pool(name="xs", bufs=4) as xs, \
             tc.tile_pool(name="acc", bufs=1) as accp:
            pm = accp.tile([128, ntiles], _F32)
            k = 0
            for n in range(nsh):
                for c in range(cinc):
                    xt = xs.tile([128, h * w], _F32, name="xt", tag="xt")
                    for q in range(nq):
                        sl = xt[:, q * hwq:(q + 1) * hwq]
                        nc.sync.dma_start(
                            sl, x.ap()[n, c * 128:(c + 1) * 128]
                            .rearrange("p a b -> p (a b)")
                            [:, q * hwq:(q + 1) * hwq])
                        nc.vector.reduce_max(pm[:, k:k + 1], sl,
                                             axis=mybir.AxisListType.X,
                                             apply_absolute_value=True)
                        k += 1
            mxt = accp.tile([128, 1], _F32)
            nc.vector.reduce_max(mxt[:], pm[:], axis=mybir.AxisListType.X)
            nc.sync.dma_start(mx.ap(), mxt[:])
    nc.compile()
    nc.m = get_hw_module(nc.m)
    return nc


def _build_conv_kernel(nsh, cin, cout, h, w):
    """Quantize x (exact fp8 pair) + 3x3 same-pad conv, fp8 DoubleRow.

    Inputs per core:
      x  [nsh, cin, h, w] f32
      wq [coc, 128, 4608] f8   (per-co-half lhsT pair blocks, partition-major)
      b  [cout//128, 128, 1] f32
      sc [128, 2] f32          (inv_beta, beta*gamma)
    Output: y [nsh, cout, h, w] f32
    """
    assert h == 56 and w == 56
    cinc, coc = cin // 128, cout // 128
    rowg = h // 8                      # 8-row output strips per image

    nc = bacc.Bacc("TRN2", target_bir_lowering=False, debug=False,
                   num_devices=_NCORES)
    x = nc.dram_tensor("x", [nsh, cin, h, w], _F32, kind="ExternalInput")
    wq = nc.dram_tensor("wq", [coc, 128, 9 * cinc * 2 * 128], _F8,
                        kind="ExternalInput")
    b = nc.dram_tensor("b", [coc, 128, 1], _F32, kind="ExternalInput")
    sc = nc.dram_tensor("sc", [128, 2], _F32, kind="ExternalInput")
    y = nc.dram_tensor("y", [nsh, cout, h, w], _F32, kind="ExternalOutput")

    Ident = mybir.ActivationFunctionType.Identity
    DR = mybir.MatmulPerfMode.DoubleRow

    with tile.TileContext(nc, trace_sim=False) as tc:
        with tc.tile_pool(name="const", bufs=1) as const, \
             tc.tile_pool(name="xstage", bufs=3) as xstage, \
             tc.tile_pool(name="outs", bufs=3) as outs, \
             tc.tile_pool(name="psum", bufs=8, space="PSUM") as psum:

            # ---- constants -------------------------------------------------
            # preload the ACT function table (lazy-load costs 1.3us on the
            # first activation otherwise)
            scratch = const.tile([128, 1], _F32)
            nc.scalar.activation(scratch[:],
                                 nc.const_aps.tensor(0.0, (128, 1)), Ident)
            # warm the PE while the head DMAs run (cost model p-state ramp)
            zw = const.tile([128, 128], _F16)
            nc.vector.memset(zw[:], 0.0)
            psw = psum.tile([128, 128], _F32, name="psw", tag="ps")
            for _ in range(_WARMUP_MMS):
                nc.tensor.matmul(psw[:], zw[:], zw[:], start=True, stop=True)

            w_sbs = [const.tile([128, 9, cinc, 2, 128], _F8,
                                name=f"w_sb{i}") for i in range(coc)]
            sc_sb = const.tile([128, 2], _F32)
            b_sb = const.tile([128, coc], _F32)
            mg_p = const.tile([128, 1], _F32)
            nc.vector.memset(mg_p[:], _M1)
            # tiny scalars lead the SWDGE queue (ACT needs sc first)
            nc.gpsimd.dma_start(sc_sb[:], sc.ap())
            nc.gpsimd.dma_start(b_sb[:], b.ap().rearrange("c p o -> p (c o)"))
            # first co-half of weights leads the DMA device (gates strip 0)
            nc.gpsimd.dma_start(w_sbs[0][:], wq.ap()[0])

            # ---- padded quantized pair planes (fp8, zero borders) ----------
            # image 0 lives in hl0 as 4 row sub-blocks (2 strips each) so the
            # first matmuls only wait for a quarter of its quantize; images
            # 1..nsh-1 are whole padded images in hl.  pair planes adjacent
            # so an MM's rhs byte-interval only spans its own image block.
            hl = const.tile([128, cinc, nsh - 1, 2, _IMG], _F8)
            hl0 = const.tile([128, cinc, len(_B0), 2, _BIMG], _F8)

            def _borders(img2, rows, top_pad, bot_pad, eng):
                """img2: [128, 2, block] pair-consolidated view."""
                if top_pad:
                    eng.memset(img2[:, :, 0:_BASE + _WP], 0.0)
                else:
                    eng.memset(img2[:, :, 0:_BASE + 1], 0.0)
                if bot_pad:
                    eng.memset(img2[:, :, _BASE + (rows - 1) * _WP:], 0.0)
                else:
                    eng.memset(img2[:, :, _BASE + rows * _WP - 1:], 0.0)
                eng.memset(
                    img2[:, :, _BASE + 57:_BASE + 57 + (rows - 1) * _WP]
                    .rearrange("p two (r t) -> p two r t", t=_WP)[:, :, :, 0:2],
                    0.0)

            # image-0 block borders up front (cheap, gate the first strips)
            mi = 0
            for c in range(cinc):
                for bi, (plo, phi) in enumerate(_B0):
                    eng = nc.vector if mi % 2 == 0 else nc.gpsimd
                    mi += 1
                    _borders(hl0[:, c, bi, :, :], phi - plo + 1,
                             plo == 0, phi == 57, eng)

            def _img2d(pair, c, n):
                return hl[:, c, n - 1, pair, _BASE:_BASE + 3364] \
                    .rearrange("p (r t) -> p r t", t=_WP)

            def _blk2d(pair, c, bi):
                plo, phi = _B0[bi]
                rows = phi - plo + 1
                return hl0[:, c, bi, pair, _BASE:_BASE + rows * _WP] \
                    .rearrange("p (r t) -> p r t", t=_WP)

            # x_q = round_half_even(x * inv_beta); |x*inv_beta| < 127 so no
            # clip needed.  Exact fp8 split: a8 = fp8(q) (RNE to <=4-bit
            # significand), b8 = q - a8 (integer in [-4,4], exact in fp8).
            qi = 0
            for n in range(nsh):
                if n >= 1:
                    # borders for this image (needed before its a/b writes)
                    for c in range(cinc):
                        eng = nc.vector if c == 0 else nc.gpsimd
                        _borders(hl[:, c, n - 1, :, :], _WP, True, True, eng)
                xts = [xstage.tile([128, h, w], _F32, name="xt", tag="xt")
                       for _ in range(cinc)]
                nch = {0: 4, 1: 2}.get(n, 1)
                rch = h // nch
                iters = []
                for r in range(0, h, rch):
                    for c in range(cinc):
                        nc.sync.dma_start(
                            xts[c][:, r:r + rch, :],
                            x.ap()[n, c * 128:(c + 1) * 128, r:r + rch, :])
                        iters.append((r, rch, c))
                    if n == 1 and r == 0:
                        nc.gpsimd.dma_start(w_sbs[1][:], wq.ap()[1])
                if True:
                    for r, rch, c in iters:
                        xsl = xts[c][:, r:r + rch, :]
                        # t = x*invb + M1 (ACT), q = t - M1 (DVE)
                        nc.scalar.activation(xsl, xsl, Ident,
                                             bias=mg_p[:],
                                             scale=sc_sb[:, 0:1])
                        nc.vector.tensor_scalar(xsl, xsl, -_M1, None,
                                                op0=mybir.AluOpType.add)
                        # destination slices: whole image (n>=1) or the
                        # sub-blocks intersecting this chunk's padrows (n=0)
                        if n == 0:
                            dests = []
                            for bi, (plo, phi) in enumerate(_B0):
                                lo = max(r + 1, plo)
                                hi = min(r + rch, phi)
                                if lo > hi:
                                    continue
                                ll, cl = lo - plo, lo - (r + 1)
                                nr = hi - lo + 1
                                dests.append((
                                    _blk2d(0, c, bi)[:, ll:ll + nr, 1:57],
                                    _blk2d(1, c, bi)[:, ll:ll + nr, 1:57],
                                    cl, nr))
                        else:
                            dests = [(
                                _img2d(0, c, n)[:, 1 + r:1 + r + rch, 1:57],
                                _img2d(1, c, n)[:, 1 + r:1 + r + rch, 1:57],
                                0, rch)]
                        for a_sl, b_sl, cl, nr in dests:
                            # a8 = fp8-RNE(q) (DVE)
                            nc.vector.tensor_scalar(
                                a_sl, xsl[:, cl:cl + nr, :], 0.0, None,
                                op0=mybir.AluOpType.add)
                            # b8 = q - a (mixed f32/fp8 in, fp8 out).
                            # Pool's t_t is ~3x slower than DVE's, so put
                            # the gate-critical image-0 c0 parts on DVE.
                            if n == 0:
                                b_eng = nc.vector if c == 0 else nc.gpsimd
                            else:
                                b_eng = nc.vector if qi % 4 == 3 \
                                    else nc.gpsimd
                            b_eng.tensor_tensor(
                                b_sl, xsl[:, cl:cl + nr, :], a_sl,
                                op=mybir.AluOpType.subtract)
                        qi += 1

            # ---- conv: 18 DoubleRow matmuls per [128co x nr x 58w] strip --
            def _mm_strip_c(ps, c, r0, co, start, stop, n=0, nr=8):
                if n == 0:
                    bi = min((r0 + 8) // 16, len(_B0) - 1)
                    lr0 = r0 - _B0[bi][0]
                for tap in range(9):
                    dh, dw = tap // 3, tap % 3
                    w_ap = w_sbs[co][:, tap, c, :, :]
                    if n == 0:
                        o = (lr0 + dh) * _WP + dw
                        rhs = hl0[:, c, bi, :, o:o + nr * _WP]
                    else:
                        o = (r0 + dh) * _WP + dw
                        rhs = hl[:, c, n - 1, :, o:o + nr * _WP]
                    nc.tensor.matmul(ps[:], w_ap, rhs,
                                     start=(start and tap == 0),
                                     stop=(stop and tap == 8),
                                     perf_mode=DR)

            def _mm_strip(ps, n, r0, co, nr=8):
                for c in range(cinc):
                    _mm_strip_c(ps, c, r0, co, start=(c == 0),
                                stop=(c == cinc - 1), n=n, nr=nr)

            for n in range(nsh):
                for co in range(coc):
                    ost = outs.tile([128, h, w], _F32, name="ost", tag="ost")
                    pss = []
                    if n == 0 and co == 0:
                        # first tile: issue all strips' c0 groups first so
                        # the PE has work while c1's quantize finishes
                        for s in range(rowg):
                            ps = psum.tile([128, 8, _WP], _F32, name="ps",
                                           tag="ps")
                            _mm_strip_c(ps, 0, 8 * s, co, start=True,
                                        stop=False)
                            pss.append(ps)
                    # the very last strip is split into 4-row halves so
                    # the post-last-matmul epilogue+DMA tail is short
                    strips = [(8 * s, 8) for s in range(rowg)]
                    if n == nsh - 1 and co == coc - 1:
                        strips = strips[:-1] + [(48, 4), (52, 4)]
                    for s, (r0, snr) in enumerate(strips):
                        if pss:
                            ps = pss[s]
                            _mm_strip_c(ps, 1, r0, co, start=False,
                                        stop=True)
                        else:
                            ps = psum.tile([128, snr, _WP], _F32, name="ps",
                                           tag="ps")
                            _mm_strip(ps, n, r0, co, nr=snr)
                        # epilogue: beta*gamma*acc + bias, drop junk
                        # cols 0 and 57; alternate ACT/DVE per strip
                        if s % 2 == 0:
                            nc.scalar.activation(ost[:, r0:r0 + snr, :],
                                                 ps[:, :, 1:57], Ident,
                                                 bias=b_sb[:, co:co + 1],
                                                 scale=sc_sb[:, 1:2])
                        else:
                            nc.vector.tensor_scalar(
                                ost[:, r0:r0 + snr, :], ps[:, :, 1:57],
                                sc_sb[:, 1:2], b_sb[:, co:co + 1],
                                op0=mybir.AluOpType.mult,
                                op1=mybir.AluOpType.add)
                    # whole-image DMAs keep HBM descriptors large (>=512B);
                    # the last image's DMAs align to strips so the final DMA
                    # covers only the last 4-row half-strip
                    if n == nsh - 1 and co == coc - 1:
                        cuts = [(8 * k, 8) for k in range(6)] + \
                               [(48, 4), (52, 4)]
                    else:
                        cuts = [(0, 28), (28, 28)]
                    for r0d, nrd in cuts:
                        nc.sync.dma_start(
                            y.ap()[n, co * 128:(co + 1) * 128,
                                   r0d:r0d + nrd, :],
                            ost[:, r0d:r0d + nrd, :])
    nc.compile()
    nc.m = get_hw_module(nc.m)
    return nc


_cache = {}


def _get(builder, *args):
    key = (builder.__name__,) + args + (tuple(sorted(_ABLATE)),)
    if key not in _cache:
        _cache[key] = builder(*args)
    return _cache[key]


def _run(nc, in_maps, cores):
    """run_bass_kernel_spmd with retries for transient device errors."""
    import time
    last = None
    for attempt in range(3):
        try:
            return run_bass_kernel_spmd(nc, in_maps, cores)
        except Exception as e:
            last = e
            time.sleep(2.0 * (attempt + 1))
    raise last


def _quantize_weights(weight, gamma):
    """Bit-exact f32 replication of the reference chimera-ternary transform."""
    f32 = np.float32
    ws = (weight / gamma).astype(f32)
    tern = np.clip(np.round(ws), f32(-1.0), f32(1.0)).astype(f32)
    raw = (f32(1.0 - 0.7) * ws + f32(0.7) * tern).astype(f32)
    # straight-through estimator is an fp identity only up to rounding:
    # replicate w + (raw - w) op-for-op, then clamp
    ste = (weight + (raw - weight)).astype(f32)
    return np.clip(ste, f32(-1.0), f32(1.0)).astype(f32)


def kernel(x, weight, bias, scale_ema):
    import ml_dtypes
    x = np.ascontiguousarray(x, dtype=np.float32)
    weight = np.ascontiguousarray(weight, dtype=np.float32)
    bias = np.ascontiguousarray(bias, dtype=np.float32)
    f32 = np.float32
    N, cin, h, w = x.shape
    cout = weight.shape[0]
    nsh = N // _NCORES
    cores = list(range(_NCORES))

    # ---- host-side tiny prep (beta-independent, done before launch 1 so
    # the gap between the two device launches is only scalar math) ---------
    gamma = np.maximum(f32(scale_ema), f32(1e-6))
    wqf = _quantize_weights(weight, gamma)
    # fp8 weights: [cout, cin, 3, 3] -> per-co-half partition-major lhsT
    # blocks [coc, ci(128), (tap, ci_chunk, pair, co_l)], pairs (w8, w8)
    w8 = wqf.astype(ml_dtypes.float8_e4m3)
    wl = np.ascontiguousarray(
        w8.transpose(2, 3, 1, 0).reshape(9, cin // 128, 128, cout))
    wp = np.stack([wl, wl], axis=3)          # [9, cinc, 128, 2, cout]
    wql = np.ascontiguousarray(
        np.stack([wp[:, :, :, :, co * 128:(co + 1) * 128]
                  .transpose(2, 0, 1, 3, 4)     # [128, 9, cinc, 2, 128]
                  .reshape(128, -1)
                  for co in range(cout // 128)], axis=0))
    b_l = np.ascontiguousarray(bias.reshape(cout // 128, 128, 1))
    ncA = _get(_build_max_kernel, nsh, cin, h, w)
    ncB = _get(_build_conv_kernel, nsh, cin, cout, h, w)

    # ---- pass 1: global abs-max -> beta ---------------------------------
    resA = _run(ncA, [{"x": x[i * nsh:(i + 1) * nsh]} for i in cores], cores)
    last_results["max"] = resA
    gmax = f32(max(f32(r["mx"].max()) for r in resA.results))
    beta = gmax / f32(127.0) + f32(1e-6)
    sc = np.tile(np.array([f32(1.0) / beta, beta * gamma], f32), (128, 1))
    sc = np.ascontiguousarray(sc)

    # ---- pass 2: quantize x + conv --------------------------------------
    in_maps = [{"x": x[i * nsh:(i + 1) * nsh], "wq": wql, "b": b_l, "sc": sc}
               for i in cores]
    resB = _run(ncB, in_maps, cores)
    last_results["conv"] = resB
    return np.concatenate([resB.results[i]["y"] for i in cores], axis=0)
